# revision 4
# baseline (speedup 1.0000x reference)
"""Trainium2 Bass kernel v4 for nn_Canvas_DIP_by_distance (vq_codebook).

v2's lean front-end (fp32 argmax chain, fp16 colors via oht-as-weights
matmuls) + baseline-style back-end (fp16 row-replication matmuls into
[128-output-rows, cols] tiles, 12 big block stores — DMA instruction count
is the scarce resource: each dma_start costs ~1us of issuing-engine time).

Per core (28 canvas rows -> 256 output rows), per w-half hf:
  sigmoid -> w4g -> v (fp32 block-diag matmul) -> one-hot (reduce+is_equal)
  -> oht (PE transposes) -> cw[w, 32ch+h] fp16 (oht-as-weights matmuls)
  -> expansion matmul vs 0/1 E -> expd16[32ch+h, sub, 512] fp16
  -> row-replication matmuls vs 0/1 rt3 -> ofs[ch][128 rows, 1024] fp32
  -> one [128, 1024] store per (ch, hf2-half, hf)
"""

import numpy as np
from contextlib import ExitStack

CANVAS_H, CANVAS_W, NUM_COLORS = 224, 224, 64
IMAGE_H = IMAGE_W = 2048
N_CORES = 8
HC = CANVAS_H // N_CORES          # 28 canvas rows per core
ORC = IMAGE_H // N_CORES          # 256 output rows per core
WH = CANVAS_W // 2                # 112
JP = HC // 2                      # 14 h-pairs

_CACHE = {}


def _build_program():
    import concourse.bacc as bacc
    import concourse.tile as tile
    import concourse.mybir as mybir

    f32 = mybir.dt.float32
    f16 = mybir.dt.float16
    ALU = mybir.AluOpType
    nc = bacc.Bacc("TRN2", target_bir_lowering=False)

    w_in = nc.dram_tensor("w_in", [HC, CANVAS_W, 3], f32, kind="ExternalInput")
    b4_in = nc.dram_tensor("b4_in", [WH, HC * 64], f32, kind="ExternalInput")
    prep2_in = nc.dram_tensor("prep2_in", [128, 6], f16, kind="ExternalInput")
    esb_in = nc.dram_tensor("esb_in", [WH, 2, 512], f16, kind="ExternalInput")
    rt3_in = nc.dram_tensor("rt3_in", [96, ORC], f16, kind="ExternalInput")
    id_in = nc.dram_tensor("id_in", [128, 128], f32, kind="ExternalInput")
    out = nc.dram_tensor("out", [3, ORC, IMAGE_W], f32, kind="ExternalOutput")

    with tile.TileContext(nc) as tc:
        with ExitStack() as ctx:
            sb = ctx.enter_context(tc.tile_pool(name="sb", bufs=1))
            ps = ctx.enter_context(tc.tile_pool(name="ps", bufs=1, space="PSUM"))

            # ---- loads ----
            wsrc = w_in[:].rearrange("h (f w) k -> w f h k", f=2)
            wraw = sb.tile([WH, 2, HC, 3], f32, tag="wraw")
            for hf in range(2):
                nc.sync.dma_start(out=wraw[:, hf], in_=wsrc[:, hf])
            ident = sb.tile([128, 128], f32, tag="ident")
            nc.sync.dma_start(out=ident[:], in_=id_in[:])
            prep2 = sb.tile([128, 6], f16, tag="prep2")
            nc.sync.dma_start(out=prep2[:], in_=prep2_in[:])
            esb = sb.tile([WH, 2, 512], f16, tag="esb")
            nc.sync.dma_start(out=esb[:], in_=esb_in[:])
            rt3 = sb.tile([96, ORC], f16, tag="rt3")
            nc.sync.dma_start(out=rt3[:], in_=rt3_in[:])
            b4 = sb.tile([WH, HC * 64], f32, tag="b4")
            nc.scalar.dma_start(out=b4[:], in_=b4_in[:])
            # fp16 identity, zero-padded to 128 cols (pads oht for FWL)
            ident16 = sb.tile([112, 128], f16, tag="ident16")
            nc.vector.tensor_copy(out=ident16[:], in_=ident[0:112, :])

            # ---- HAM warmup: ~4us of dummy matmuls while DMAs load, so
            # the PE clock gate opens (1.2 -> 2.4 GHz) before real work ----
            for i in range(12):
                wp = ps.tile([128, 512], f32, tag="psR", bufs=2)
                nc.tensor.matmul(
                    out=wp[0:112, 0:128], lhsT=ident[0:112, 0:112],
                    rhs=ident[0:112, :], start=True, stop=True)

            # ---- sigmoid + augmented ones column ----
            waug = sb.tile([WH, 2, HC, 4], f32, tag="waug")
            for hf in range(2):
                nc.scalar.activation(
                    out=waug[:, hf, :, 0:3], in_=wraw[:, hf],
                    func=mybir.ActivationFunctionType.Sigmoid)
            nc.vector.memset(waug[:, :, :, 3:4], 1.0)

            ofs = [sb.tile([128, 2, 1024], f32, tag=f"ofs{ch}",
                           name=f"ofs{ch}") for ch in range(3)]

            for hf in range(2):
                # ---- w4g[(h,k), w] via fp32 PE transpose ----
                tp = ps.tile([WH, 512], f32, tag="psS", bufs=2)
                nc.tensor.transpose(
                    out=tp[:, 0:112],
                    in_=waug[:, hf].rearrange("w h k -> w (h k)"),
                    identity=ident[0:WH, 0:112])
                w4g = sb.tile([112, WH], f32, tag=f"w4g{hf}")
                nc.scalar.copy(out=w4g[:], in_=tp[:, 0:112])

                # ---- v + argmax one-hot ----
                oh = sb.tile([WH, HC, NUM_COLORS], f16, tag=f"oh{hf}")
                for g in range(4):
                    nh = min(8, HC - 8 * g)
                    nn = 64 * nh
                    sp = ps.tile([WH, 512], f32, tag="psS", bufs=2)
                    nc.tensor.matmul(
                        out=sp[:, 0:nn], lhsT=w4g[:],
                        rhs=b4[:, 512 * g:512 * g + nn],
                        start=True, stop=True)
                    spv = sp[:, 0:nn].rearrange("w (h c) -> w h c", c=64)
                    vm = sb.tile([WH, 8], f32, tag="vmax", bufs=2)
                    nc.vector.tensor_reduce(
                        out=vm[:, 0:nh], in_=spv, axis=mybir.AxisListType.X,
                        op=ALU.max)
                    nc.vector.tensor_tensor(
                        out=oh[:, 8 * g:8 * g + nh], in0=spv,
                        in1=vm[:, 0:nh].unsqueeze(2).to_broadcast([WH, nh, 64]),
                        op=ALU.is_equal)

                # ---- transpose one-hot: oht[(dh,c), j', 0:112]=oh.T ----
                oht = sb.tile([128, JP, 128], f16, tag=f"oht{hf}")
                for half in range(2):
                    j0 = 7 * half
                    tp16 = ps.tile([128, 7, 128], f16, tag="psB", bufs=2)
                    for j in range(j0, j0 + 7):
                        nc.tensor.transpose(
                            out=tp16[:, j - j0, :],
                            in_=oh[:, 2 * j:2 * j + 2]
                            .rearrange("w h c -> w (h c)"),
                            identity=ident16[:])
                    dst = oht[:, j0:j0 + 7].rearrange("c j w -> c (j w)")
                    if half == 0:
                        nc.vector.tensor_copy(
                            out=dst, in_=tp16[:].rearrange("c j w -> c (j w)"))
                    else:
                        nc.scalar.copy(
                            out=dst, in_=tp16[:].rearrange("c j w -> c (j w)"))

                # ---- colors, w-partitioned: cw[w, 32ch + 2j'+dh] fp16 ----
                cw = sb.tile([WH, 96], f16, tag=f"cw{hf}")
                nc.vector.memset(cw[:], 0.0)
                cp = ps.tile([128, JP, 9], f32, tag="psC", bufs=1)
                for j in range(JP):
                    nc.tensor.matmul(
                        out=cp[:, j, 0:6], lhsT=oht[:, j, :],
                        rhs=prep2[:], start=True, stop=True)
                # scatter (j', dh, ch) -> col 32ch + 2j' + dh
                src4 = (cp[0:WH, :, 0:6]
                        .rearrange("w j (d c) -> w j d c", d=2))
                dst4 = (cw[:].rearrange("w (c x) -> w c x", c=3)[:, :, 0:28]
                        .rearrange("w c (j d) -> w j d c", d=2))
                nc.scalar.copy(out=dst4, in_=src4)

                # ---- column expansion -> expd16 fp16 [32ch+h, sub, 512] --
                expd16 = sb.tile([96, 2, 512], f16, tag=f"expd{hf}")
                for sub in range(2):
                    ep = ps.tile([96, 512], f32, tag="psE", bufs=1)
                    nc.tensor.matmul(
                        out=ep[:], lhsT=cw[:], rhs=esb[:, sub, :],
                        start=True, stop=True)
                    if sub == 0:
                        nc.vector.tensor_copy(out=expd16[:, sub, :], in_=ep[:])
                    else:
                        nc.scalar.copy(out=expd16[:, sub, :], in_=ep[:])

                # ---- row replication matmuls + block stores ----
                for hf2 in range(2):
                    for ch in range(3):
                        for sub in range(2):
                            rp = ps.tile([128, 512], f32, tag="psR", bufs=2)
                            nc.tensor.matmul(
                                out=rp[:],
                                lhsT=rt3[32 * ch:32 * ch + 28,
                                         128 * hf2:128 * hf2 + 128],
                                rhs=expd16[32 * ch:32 * ch + 28, sub, :],
                                start=True, stop=True)
                            dst = ofs[ch][:, hf2, 512 * sub:512 * sub + 512]
                            if (ch + sub) % 2 == 0:
                                nc.vector.tensor_copy(out=dst, in_=rp[:])
                            else:
                                nc.scalar.copy(out=dst, in_=rp[:])
                    for ch in range(3):
                        eng = nc.sync if (hf == 0 or ch != 2) else nc.scalar
                        eng.dma_start(
                            out=out[ch, 128 * hf2:128 * hf2 + 128,
                                    1024 * hf:1024 * hf + 1024],
                            in_=ofs[ch][:, hf2])

    nc.compile()
    return nc


def _host_consts(palette: np.ndarray):
    pal = palette.astype(np.float32)
    # block-diagonal distance matrix: rows (28h x 4k), cols (28h x 64c)
    b4row = np.empty((4, NUM_COLORS), np.float32)
    b4row[0:3] = -pal.T
    b4row[3] = 0.5 * (pal.astype(np.float64) ** 2).sum(-1).astype(np.float32)
    b4 = np.zeros((WH, HC * NUM_COLORS), np.float32)
    for h in range(HC):
        b4[4 * h:4 * h + 4, 64 * h:64 * h + 64] = b4row
    # palette fp16, block-diagonal over dh: prep2[64dh+c, 3dh+ch]
    pal16 = pal.astype(np.float16)
    prep2 = np.zeros((128, 6), np.float16)
    prep2[0:64, 0:3] = pal16
    prep2[64:128, 3:6] = pal16
    # zero-padded local column-expansion matrices (0/1, fp16-exact)
    wmap = (np.arange(512) * CANVAS_W) // IMAGE_W
    e_loc = (wmap[None, :] == np.arange(56)[:, None]).astype(np.float16)
    esb = np.zeros((WH, 2, 512), np.float16)
    esb[0:56, 0] = e_loc
    esb[56:112, 1] = e_loc
    # row-replication matrix: rt3[32ch + h, r] = (rowmap(r) == h)
    rowmap = (np.arange(ORC) * CANVAS_H) // IMAGE_H
    rt = (rowmap[None, :] == np.arange(32)[:, None]).astype(np.float16)
    rt3 = np.concatenate([rt, rt, rt], axis=0)       # (96, 256)
    ident = np.eye(128, dtype=np.float32)
    return b4, prep2, esb, rt3, ident


def kernel(weight_logits, palette, image_h, image_w):
    weight_logits = np.asarray(weight_logits, np.float32)
    palette = np.asarray(palette, np.float32)
    assert int(image_h) == IMAGE_H and int(image_w) == IMAGE_W
    assert weight_logits.shape == (CANVAS_H, CANVAS_W, 3)

    if "nc" not in _CACHE:
        _CACHE["nc"] = _build_program()
    nc = _CACHE["nc"]

    from concourse import bass_utils

    b4, prep2, esb, rt3, ident = _host_consts(palette)
    in_maps = []
    for core in range(N_CORES):
        sl = weight_logits[core * HC:(core + 1) * HC]
        in_maps.append({
            "w_in": np.ascontiguousarray(sl),
            "b4_in": b4, "prep2_in": prep2, "esb_in": esb,
            "rt3_in": rt3, "id_in": ident,
        })
    res = bass_utils.run_bass_kernel_spmd(
        nc, in_maps, core_ids=list(range(N_CORES)))
    outs = [res.results[c]["out"] for c in range(N_CORES)]
    return np.concatenate(outs, axis=1)


# revision 5
# speedup vs baseline: 1.1125x; 1.1125x over previous
"""Trainium2 Bass kernel v4 for nn_Canvas_DIP_by_distance (vq_codebook).

v2's lean front-end (fp32 argmax chain, fp16 colors via oht-as-weights
matmuls) + baseline-style back-end (fp16 row-replication matmuls into
[128-output-rows, cols] tiles, 12 big block stores — DMA instruction count
is the scarce resource: each dma_start costs ~1us of issuing-engine time).

Per core (28 canvas rows -> 256 output rows), per w-half hf:
  sigmoid -> w4g -> v (fp32 block-diag matmul) -> one-hot (reduce+is_equal)
  -> oht (PE transposes) -> cw[w, 32ch+h] fp16 (oht-as-weights matmuls)
  -> expansion matmul vs 0/1 E -> expd16[32ch+h, sub, 512] fp16
  -> row-replication matmuls vs 0/1 rt3 -> ofs[ch][128 rows, 1024] fp32
  -> one [128, 1024] store per (ch, hf2-half, hf)
"""

import numpy as np
from contextlib import ExitStack

CANVAS_H, CANVAS_W, NUM_COLORS = 224, 224, 64
IMAGE_H = IMAGE_W = 2048
N_CORES = 8
HC = CANVAS_H // N_CORES          # 28 canvas rows per core
ORC = IMAGE_H // N_CORES          # 256 output rows per core
WH = CANVAS_W // 2                # 112
JP = HC // 2                      # 14 h-pairs

_CACHE = {}


def _build_program():
    import concourse.bacc as bacc
    import concourse.tile as tile
    import concourse.mybir as mybir

    f32 = mybir.dt.float32
    f16 = mybir.dt.float16
    ALU = mybir.AluOpType
    nc = bacc.Bacc("TRN2", target_bir_lowering=False)

    w_in = nc.dram_tensor("w_in", [HC, CANVAS_W, 3], f32, kind="ExternalInput")
    b4_in = nc.dram_tensor("b4_in", [WH, HC * 64], f32, kind="ExternalInput")
    prep2_in = nc.dram_tensor("prep2_in", [128, 6], f16, kind="ExternalInput")
    esb_in = nc.dram_tensor("esb_in", [WH, 2, 512], f16, kind="ExternalInput")
    rt3_in = nc.dram_tensor("rt3_in", [96, ORC], f16, kind="ExternalInput")
    id_in = nc.dram_tensor("id_in", [128, 128], f32, kind="ExternalInput")
    out = nc.dram_tensor("out", [3, ORC, IMAGE_W], f32, kind="ExternalOutput")

    with tile.TileContext(nc) as tc:
        with ExitStack() as ctx:
            sb = ctx.enter_context(tc.tile_pool(name="sb", bufs=1))
            ps = ctx.enter_context(tc.tile_pool(name="ps", bufs=1, space="PSUM"))

            # ---- loads ----
            wsrc = w_in[:].rearrange("h (f w) k -> w f h k", f=2)
            wraw = sb.tile([WH, 2, HC, 3], f32, tag="wraw")
            for hf in range(2):
                nc.sync.dma_start(out=wraw[:, hf], in_=wsrc[:, hf])
            ident = sb.tile([128, 128], f32, tag="ident")
            nc.sync.dma_start(out=ident[:], in_=id_in[:])
            prep2 = sb.tile([128, 6], f16, tag="prep2")
            nc.sync.dma_start(out=prep2[:], in_=prep2_in[:])
            esb = sb.tile([WH, 2, 512], f16, tag="esb")
            nc.sync.dma_start(out=esb[:], in_=esb_in[:])
            rt3 = sb.tile([96, ORC], f16, tag="rt3")
            nc.sync.dma_start(out=rt3[:], in_=rt3_in[:])
            b4 = sb.tile([WH, HC * 64], f32, tag="b4")
            nc.scalar.dma_start(out=b4[:], in_=b4_in[:])
            # fp16 identity, zero-padded to 128 cols (pads oht for FWL)
            ident16 = sb.tile([112, 128], f16, tag="ident16")
            nc.vector.tensor_copy(out=ident16[:], in_=ident[0:112, :])

            # ---- sigmoid + augmented ones column ----
            waug = sb.tile([WH, 2, HC, 4], f32, tag="waug")
            for hf in range(2):
                nc.scalar.activation(
                    out=waug[:, hf, :, 0:3], in_=wraw[:, hf],
                    func=mybir.ActivationFunctionType.Sigmoid)
            nc.vector.memset(waug[:, :, :, 3:4], 1.0)

            ofs = [sb.tile([128, 2, 1024], f32, tag=f"ofs{ch}",
                           name=f"ofs{ch}") for ch in range(3)]

            for hf in range(2):
                # ---- w4g[(h,k), w] via fp32 PE transpose ----
                tp = ps.tile([WH, 512], f32, tag="psS", bufs=2)
                nc.tensor.transpose(
                    out=tp[:, 0:112],
                    in_=waug[:, hf].rearrange("w h k -> w (h k)"),
                    identity=ident[0:WH, 0:112])
                w4g = sb.tile([112, WH], f32, tag=f"w4g{hf}")
                nc.scalar.copy(out=w4g[:], in_=tp[:, 0:112])

                # ---- v + argmax one-hot ----
                oh = sb.tile([WH, HC, NUM_COLORS], f16, tag=f"oh{hf}")
                for g in range(4):
                    nh = min(8, HC - 8 * g)
                    nn = 64 * nh
                    sp = ps.tile([WH, 512], f32, tag="psS", bufs=2)
                    nc.tensor.matmul(
                        out=sp[:, 0:nn], lhsT=w4g[:],
                        rhs=b4[:, 512 * g:512 * g + nn],
                        start=True, stop=True)
                    spv = sp[:, 0:nn].rearrange("w (h c) -> w h c", c=64)
                    vm = sb.tile([WH, 8], f32, tag="vmax", bufs=2)
                    nc.vector.tensor_reduce(
                        out=vm[:, 0:nh], in_=spv, axis=mybir.AxisListType.X,
                        op=ALU.max)
                    nc.vector.tensor_tensor(
                        out=oh[:, 8 * g:8 * g + nh], in0=spv,
                        in1=vm[:, 0:nh].unsqueeze(2).to_broadcast([WH, nh, 64]),
                        op=ALU.is_equal)

                # ---- transpose one-hot: oht[(dh,c), j', 0:112]=oh.T ----
                oht = sb.tile([128, JP, 128], f16, tag=f"oht{hf}")
                for half in range(2):
                    j0 = 7 * half
                    tp16 = ps.tile([128, 7, 128], f16, tag="psB", bufs=2)
                    for j in range(j0, j0 + 7):
                        nc.tensor.transpose(
                            out=tp16[:, j - j0, :],
                            in_=oh[:, 2 * j:2 * j + 2]
                            .rearrange("w h c -> w (h c)"),
                            identity=ident16[:])
                    dst = oht[:, j0:j0 + 7].rearrange("c j w -> c (j w)")
                    if half == 0:
                        nc.vector.tensor_copy(
                            out=dst, in_=tp16[:].rearrange("c j w -> c (j w)"))
                    else:
                        nc.scalar.copy(
                            out=dst, in_=tp16[:].rearrange("c j w -> c (j w)"))

                # ---- colors, w-partitioned: cw[w, 32ch + 2j'+dh] fp16 ----
                cw = sb.tile([WH, 96], f16, tag=f"cw{hf}")
                nc.vector.memset(cw[:], 0.0)
                cp = ps.tile([128, JP, 9], f32, tag="psC", bufs=1)
                for j in range(JP):
                    nc.tensor.matmul(
                        out=cp[:, j, 0:6], lhsT=oht[:, j, :],
                        rhs=prep2[:], start=True, stop=True)
                # scatter (j', dh, ch) -> col 32ch + 2j' + dh
                src4 = (cp[0:WH, :, 0:6]
                        .rearrange("w j (d c) -> w j d c", d=2))
                dst4 = (cw[:].rearrange("w (c x) -> w c x", c=3)[:, :, 0:28]
                        .rearrange("w c (j d) -> w j d c", d=2))
                nc.scalar.copy(out=dst4, in_=src4)

                # ---- column expansion -> expd16 fp16 [32ch+h, sub, 512] --
                expd16 = sb.tile([96, 2, 512], f16, tag=f"expd{hf}")
                for sub in range(2):
                    ep = ps.tile([96, 512], f32, tag="psE", bufs=1)
                    nc.tensor.matmul(
                        out=ep[:], lhsT=cw[:], rhs=esb[:, sub, :],
                        start=True, stop=True)
                    if sub == 0:
                        nc.vector.tensor_copy(out=expd16[:, sub, :], in_=ep[:])
                    else:
                        nc.scalar.copy(out=expd16[:, sub, :], in_=ep[:])

                # ---- row replication matmuls + block stores ----
                for hf2 in range(2):
                    for ch in range(3):
                        for sub in range(2):
                            rp = ps.tile([128, 512], f32, tag="psR", bufs=2)
                            nc.tensor.matmul(
                                out=rp[:],
                                lhsT=rt3[32 * ch:32 * ch + 28,
                                         128 * hf2:128 * hf2 + 128],
                                rhs=expd16[32 * ch:32 * ch + 28, sub, :],
                                start=True, stop=True)
                            dst = ofs[ch][:, hf2, 512 * sub:512 * sub + 512]
                            if (ch + sub) % 2 == 0:
                                nc.vector.tensor_copy(out=dst, in_=rp[:])
                            else:
                                nc.scalar.copy(out=dst, in_=rp[:])
                    for ch in range(3):
                        eng = nc.sync if (hf == 0 or ch != 2) else nc.scalar
                        eng.dma_start(
                            out=out[ch, 128 * hf2:128 * hf2 + 128,
                                    1024 * hf:1024 * hf + 1024],
                            in_=ofs[ch][:, hf2])

    nc.compile()
    return nc


def _host_consts(palette: np.ndarray):
    pal = palette.astype(np.float32)
    # block-diagonal distance matrix: rows (28h x 4k), cols (28h x 64c)
    b4row = np.empty((4, NUM_COLORS), np.float32)
    b4row[0:3] = -pal.T
    b4row[3] = 0.5 * (pal.astype(np.float64) ** 2).sum(-1).astype(np.float32)
    b4 = np.zeros((WH, HC * NUM_COLORS), np.float32)
    for h in range(HC):
        b4[4 * h:4 * h + 4, 64 * h:64 * h + 64] = b4row
    # palette fp16, block-diagonal over dh: prep2[64dh+c, 3dh+ch]
    pal16 = pal.astype(np.float16)
    prep2 = np.zeros((128, 6), np.float16)
    prep2[0:64, 0:3] = pal16
    prep2[64:128, 3:6] = pal16
    # zero-padded local column-expansion matrices (0/1, fp16-exact)
    wmap = (np.arange(512) * CANVAS_W) // IMAGE_W
    e_loc = (wmap[None, :] == np.arange(56)[:, None]).astype(np.float16)
    esb = np.zeros((WH, 2, 512), np.float16)
    esb[0:56, 0] = e_loc
    esb[56:112, 1] = e_loc
    # row-replication matrix: rt3[32ch + h, r] = (rowmap(r) == h)
    rowmap = (np.arange(ORC) * CANVAS_H) // IMAGE_H
    rt = (rowmap[None, :] == np.arange(32)[:, None]).astype(np.float16)
    rt3 = np.concatenate([rt, rt, rt], axis=0)       # (96, 256)
    ident = np.eye(128, dtype=np.float32)
    return b4, prep2, esb, rt3, ident


def kernel(weight_logits, palette, image_h, image_w):
    weight_logits = np.asarray(weight_logits, np.float32)
    palette = np.asarray(palette, np.float32)
    assert int(image_h) == IMAGE_H and int(image_w) == IMAGE_W
    assert weight_logits.shape == (CANVAS_H, CANVAS_W, 3)

    if "nc" not in _CACHE:
        _CACHE["nc"] = _build_program()
    nc = _CACHE["nc"]

    from concourse import bass_utils

    b4, prep2, esb, rt3, ident = _host_consts(palette)
    in_maps = []
    for core in range(N_CORES):
        sl = weight_logits[core * HC:(core + 1) * HC]
        in_maps.append({
            "w_in": np.ascontiguousarray(sl),
            "b4_in": b4, "prep2_in": prep2, "esb_in": esb,
            "rt3_in": rt3, "id_in": ident,
        })
    res = bass_utils.run_bass_kernel_spmd(
        nc, in_maps, core_ids=list(range(N_CORES)))
    outs = [res.results[c]["out"] for c in range(N_CORES)]
    return np.concatenate(outs, axis=1)
